# revision 24
# baseline (speedup 1.0000x reference)
"""AFT (attention-free transformer) block on 8 TRN2 NeuronCores.

Reference computation (T=2048, B=4, D=1024):
    qkv = data @ W_qkv + b_qkv ; q,k,v = split(qkv)
    num = exp(pb - max_pb) @ (exp(k - max_k) * v)    (contraction over key pos j)
    den = exp(pb - max_pb) @ exp(k - max_k)
    out = (sigmoid(q) * num / den) @ W_out + b_out
The max shifts cancel exactly in num/den so the kernel drops them.

Sharding: hybrid (sequence-half x batch). Core r = 2b + h owns batch b and
query rows i in [h*1024, (h+1)*1024). Each core projects q/k/v for its own
1024 tokens; k-half exchange is a PAIRWISE AllGather (replica groups
[[0,1],[2,3],[4,5],[6,7]]) of fp8 exp(k) / exp(k)*v, two pipelined chunks.

Precision trick: exp(pb) = 1 + r with r = expm1(pb) in [-0.09, 0.1], so
    num = Snum + r @ ekv,   Snum[d] = sum_j ekv[j,d]   (i-independent)
    den = Sden + r @ ek
The S sums are computed in bf16/fp32 (ones-matmul on the PE, then a
K=1-matmul transpose into per-partition columns); the big TxT einsum runs
on the small correction term with BOTH operands fp8e4 and
perf_mode=DoubleRow (K virtualized to 256, ~2x PE throughput). The fp8
quantization error only touches the ~2% correction, keeping overall rel
err ~5e-3. r is pre-scaled by 64 on the host (epilogue rescales by 1/64).

Everything downstream of the pb einsum is TRANSPOSED ([d,i] layout): the q
projection emits sigmoid(q)^T directly (lhsT = W_q), num/den come out of
the DoubleRow matmuls as [d_chunk, i], and the output projection consumes
y^T as lhsT directly -- no PE transposes, no spill/merge passes.

Timeline per core: kv projection (8 token tiles, chunk AGs fired at tiles
3/7) -> S finalize -> q^T projection + sigmoid (covers AG wire time) ->
num/den DoubleRow accumulation (16 j-tiles as 8 pairs, single PSUM pass)
-> epilogue (reciprocal, sigmoid multiply) -> output projection.
"""

import numpy as np
import ml_dtypes

from concourse import bacc, bass, mybir, tile
from concourse.bass_utils import run_bass_kernel_spmd

BF16 = mybir.dt.bfloat16
F32 = mybir.dt.float32
F8 = mybir.dt.float8e4
AF = mybir.ActivationFunctionType
ALU = mybir.AluOpType
DR = mybir.MatmulPerfMode.DoubleRow

N_CORES = 8
T, B, D = 2048, 4, 1024
TOK = 1024                 # tokens per core: 1024 query rows of one batch
KT = D // 128              # 8 contraction tiles for d_in
NG = TOK // 128            # 8 token/query tile groups
PAIRS = T // 256           # 8 j-block pairs (DoubleRow processes 256 j rows)
SCALE = 64.0               # host pre-scale on expm1(pb) for fp8 range
PAIR_GROUPS = [[0, 1], [2, 3], [4, 5], [6, 7]]
# AG chunking over the 8 own token tiles: small first chunk fires the
# wire early; small last chunk keeps the serial collective tail short.
CH_TILES = [2, 4, 2]
CH_START = [0, 2, 6]
CH_PAIRS = [1, 2, 1]
# nd pair order = chunk-major (earliest-gathered first)
U_LIST = [(x, hp, q) for x in range(3) for hp in range(2)
          for q in range(CH_PAIRS[x])]

_cache = {}


def build(with_qkv_bias: bool, with_out_bias: bool):
    nc = bacc.Bacc(None, target_bir_lowering=False)

    dataT_d = nc.dram_tensor("dataT", [D, TOK], BF16, kind="ExternalInput")
    wkv_d = nc.dram_tensor("wkv", [D, 2 * D], BF16, kind="ExternalInput")
    wq_d = nc.dram_tensor("wq", [D, D], BF16, kind="ExternalInput")
    pbr_d = nc.dram_tensor("pbr", [TOK, 2048], F8, kind="ExternalInput")
    wout_d = nc.dram_tensor("wout", [D, D], BF16, kind="ExternalInput")
    out_d = nc.dram_tensor("out", [TOK, D], F32, kind="ExternalOutput")
    if with_qkv_bias:
        bkv_d = nc.dram_tensor("bkv", [1, 2 * D], BF16, kind="ExternalInput")
        bqt_d = nc.dram_tensor("bqt", [128, KT], F32, kind="ExternalInput")
    if with_out_bias:
        bout_d = nc.dram_tensor("bout", [1, D], BF16, kind="ExternalInput")

    with tile.TileContext(nc) as tc:
        with (
            tc.tile_pool(name="persist", bufs=1) as pp,
            tc.tile_pool(name="psum", bufs=6, space="PSUM") as psp,
            tc.tile_pool(name="psum_s", bufs=1, space="PSUM") as pss,
            tc.tile_pool(name="dram", bufs=1, space="DRAM") as dram,
        ):
            # ---- persistent SBUF tensors ----
            onescol = pp.tile([128, 1], BF16, name="onescol", tag="onescol")
            nc.gpsimd.memset(onescol[:], 1.0)
            # 64.0 as the K=1 matmul rhs: ST holds 64*S so the 1/SCALE on
            # the einsum and the S bias cancel exactly in num/den
            c64 = pp.tile([1, 1], F32, name="c64", tag="c64")
            nc.gpsimd.memset(c64[:], SCALE)
            # ST cols 0-7: Sden per d-chunk; cols 8-15: Snum
            ST = pp.tile([128, 16], F32, name="ST", tag="ST")
            srow_d = pp.tile([1, D], F32, name="srow_d", tag="srow_d")
            srow_n = pp.tile([1, D], F32, name="srow_n", tag="srow_n")
            sr2_d = pp.tile([1, D], F32, name="sr2_d", tag="sr2_d")
            sr2_n = pp.tile([1, D], F32, name="sr2_n", tag="sr2_n")
            wout = [pp.tile([128, D], BF16, name=f"wout{k}", tag=f"wout{k}")
                    for k in range(KT)]
            pbr = [pp.tile([128, 2, TOK], F8, name=f"pbr{u}", tag=f"pbr{u}")
                   for u in range(PAIRS)]
            sq_t = [pp.tile([128, TOK], BF16, name=f"sq{c}", tag=f"sq{c}")
                    for c in range(KT)]
            # gathered fp8 j-pair tiles live in the persistent pool: if they
            # were allocated in the phase-B pool they would reuse dataT/wq
            # addresses and their DMAs would stall on a WAR hazard until the
            # q projection finishes reading those tiles (~25us late).
            ekg = [pp.tile([128, 2, TOK], F8, name=f"ekg{u}", tag=f"ekg{u}")
                   for u in range(PAIRS)]
            ekvg = [pp.tile([128, 2, TOK], F8, name=f"ekvg{u}",
                            tag=f"ekvg{u}") for u in range(PAIRS)]
            if with_qkv_bias or with_out_bias:
                ones1r = pp.tile([1, 128], BF16, name="ones1r", tag="ones1r")
                nc.gpsimd.memset(ones1r[:], 1.0)
            if with_qkv_bias:
                bkv = pp.tile([1, 2 * D], BF16, name="bkv", tag="bkv")
                nc.sync.dma_start(bkv[:], bkv_d[:])
                bqt = pp.tile([128, KT], F32, name="bqt", tag="bqt")
                nc.sync.dma_start(bqt[:], bqt_d[:])
            if with_out_bias:
                bout = pp.tile([1, D], BF16, name="bout", tag="bout")
                nc.sync.dma_start(bout[:], bout_d[:])

            # S accumulators: row 0 = Sden, row 32 = Snum (PE col-strips)
            s2 = [pss.tile([64, 512], F32, name=f"s2_{ih}", tag=f"s2_{ih}")
                  for ih in range(2)]

            # collective bounce buffers, fp8, one chunk per 512 own tokens:
            # rows 0:256 = ek pairs (q=0,1), 256:512 = ekv pairs; within a
            # pair row-block the two 128-j subtiles sit in column halves
            # (the [128, 2, 1024] DoubleRow layout).
            # the last chunk carries 4 extra rows: the core's own-half S
            # sums as raw fp32 bytes (2 rows Sden, 2 rows Snum); both
            # halves' rows come back with the gather and are added on-chip,
            # so no separate AllReduce is needed.
            cc_rows = [2 * CH_PAIRS[x] * 128 + (4 if x == 2 else 0)
                       for x in range(3)]
            cc_in = [dram.tile([cc_rows[x], 2048], F8, name=f"cc_in{x}")
                     for x in range(3)]
            cc_out = [dram.tile([2 * cc_rows[x], 2048], F8, name=f"cc_out{x}")
                      for x in range(3)]

            # ---- phase A: kv projection -> fp8 staging -> pairwise AG ----
            with tc.tile_pool(name="phaseA", bufs=1) as pa:
                dataT = [pa.tile([128, TOK], BF16, name=f"dataT{k}",
                                 tag=f"dataT{k}") for k in range(KT)]
                wkv = [pa.tile([128, 2 * D], BF16, name=f"wkv{k}",
                               tag=f"wkv{k}") for k in range(KT)]
                wq = [pa.tile([128, D], BF16, name=f"wq{k}", tag=f"wq{k}")
                      for k in range(KT)]
                for k in range(KT):
                    nc.sync.dma_start(dataT[k][:], dataT_d[k * 128:(k + 1) * 128, :])
                    nc.sync.dma_start(wkv[k][:, :D],
                                      wkv_d[k * 128:(k + 1) * 128, :D])
                    nc.sync.dma_start(wkv[k][:, D:],
                                      wkv_d[k * 128:(k + 1) * 128, D:])
                for k in range(KT):
                    nc.sync.dma_start(wq[k][:], wq_d[k * 128:(k + 1) * 128, :])
                for u in range(PAIRS):
                    nc.sync.dma_start(pbr[u][:], pbr_d[u * 128:(u + 1) * 128, :])
                for k in range(KT):
                    nc.sync.dma_start(wout[k][:], wout_d[k * 128:(k + 1) * 128, :])

                for m in range(NG):
                    ps = [psp.tile([128, 512], F32, name=f"ps{m}_{i}",
                                   tag="mm") for i in range(4)]
                    for k in range(KT):
                        for i in range(4):
                            nc.tensor.matmul(
                                ps[i][:], dataT[k][:, m * 128:(m + 1) * 128],
                                wkv[k][:, i * 512:(i + 1) * 512],
                                start=(k == 0),
                                stop=(k == KT - 1 and not with_qkv_bias),
                            )
                    if with_qkv_bias:
                        for i in range(4):
                            nc.tensor.matmul(
                                ps[i][:], ones1r[:], bkv[:, i * 512:(i + 1) * 512],
                                start=False, stop=True,
                            )
                    ek = pa.tile([128, D], BF16, name=f"ek{m}", tag="ek", bufs=3)
                    ekv = pa.tile([128, D], BF16, name=f"ekv{m}", tag="ekv",
                                  bufs=3)
                    for ih in range(2):
                        sl = slice(ih * 512, (ih + 1) * 512)
                        nc.scalar.activation(ek[:, sl], ps[ih][:], AF.Exp)
                        nc.vector.tensor_mul(ekv[:, sl], ek[:, sl], ps[2 + ih][:])
                        # S sums over this tile's 128 j rows (bf16 source,
                        # fp32 accum): row 0 <- ek, row 32 <- ekv
                        nc.tensor.matmul(
                            s2[ih][0:1, :], onescol[:], ek[:, sl],
                            start=(m == 0), stop=(m == NG - 1),
                            skip_group_check=True)
                        nc.tensor.matmul(
                            s2[ih][32:33, :], onescol[:], ekv[:, sl],
                            start=(m == 0), stop=(m == NG - 1),
                            skip_group_check=True)
                    ek8 = pa.tile([128, D], F8, name=f"ek8{m}", tag="ek8",
                                  bufs=3)
                    ekv8 = pa.tile([128, D], F8, name=f"ekv8{m}", tag="ekv8",
                                   bufs=3)
                    nc.vector.tensor_copy(ek8[:], ek[:])
                    nc.vector.tensor_copy(ekv8[:], ekv[:])
                    x = 0 if m < 2 else (1 if m < 6 else 2)
                    l = m - CH_START[x]
                    q, t = l // 2, l % 2
                    ekv_base = CH_PAIRS[x] * 128
                    nc.sync.dma_start(
                        cc_in[x][q * 128:(q + 1) * 128,
                                 t * 1024:(t + 1) * 1024], ek8[:])
                    nc.sync.dma_start(
                        cc_in[x][ekv_base + q * 128:ekv_base + (q + 1) * 128,
                                 t * 1024:(t + 1) * 1024], ekv8[:])
                    if m in (1, 5):
                        x = 0 if m == 1 else 1
                        nc.gpsimd.collective_compute(
                            "AllGather", ALU.bypass,
                            replica_groups=PAIR_GROUPS,
                            ins=[cc_in[x][:].opt()],
                            outs=[cc_out[x][:].opt()],
                        )

                # S finalize: PSUM rows -> fp32 SBUF rows -> packed as raw
                # bytes into the last chunk's tail rows, shipped by its AG
                for ih in range(2):
                    sl = slice(ih * 512, (ih + 1) * 512)
                    nc.scalar.copy(srow_d[0:1, sl], s2[ih][0:1, :])
                    nc.scalar.copy(srow_n[0:1, sl], s2[ih][32:33, :])
                sb = 2 * CH_PAIRS[2] * 128  # S rows base in cc_in[2]
                for ih in range(2):
                    sl = slice(ih * 512, (ih + 1) * 512)
                    nc.sync.dma_start(cc_in[2][sb + ih:sb + 1 + ih, :],
                                      srow_d[:, sl].bitcast(F8))
                    nc.sync.dma_start(cc_in[2][sb + 2 + ih:sb + 3 + ih, :],
                                      srow_n[:, sl].bitcast(F8))
                nc.gpsimd.collective_compute(
                    "AllGather", ALU.bypass,
                    replica_groups=PAIR_GROUPS,
                    ins=[cc_in[2][:].opt()],
                    outs=[cc_out[2][:].opt()],
                )

                # q^T projection + sigmoid (overlaps the collectives)
                for c in range(KT):
                    psq = [psp.tile([128, 512], F32, name=f"psq{c}_{ih}",
                                    tag="mm") for ih in range(2)]
                    for k in range(KT):
                        for ih in range(2):
                            nc.tensor.matmul(
                                psq[ih][:], wq[k][:, c * 128:(c + 1) * 128],
                                dataT[k][:, ih * 512:(ih + 1) * 512],
                                start=(k == 0), stop=(k == KT - 1),
                            )
                    for ih in range(2):
                        sl = slice(ih * 512, (ih + 1) * 512)
                        if with_qkv_bias:
                            nc.scalar.activation(
                                sq_t[c][:, sl], psq[ih][:], AF.Sigmoid,
                                bias=bqt[:, c:c + 1])
                        else:
                            nc.scalar.activation(
                                sq_t[c][:, sl], psq[ih][:], AF.Sigmoid)

                def emit_s_transpose():
                    # 16 micro-MMs turning the summed S rows into
                    # per-partition bias columns, scaled by 64 (rhs = c64).
                    # They wait on AG chunk 1, so they are emitted BEHIND
                    # the first num/den MM block in the in-order PE queue
                    # (the MMs don't need ST; only the epilogue does).
                    stp = psp.tile([128, 512], F32, name="stp", tag="mm")
                    for c in range(KT):
                        nc.tensor.matmul(
                            stp[:, c:c + 1],
                            sr2_d[0:1, c * 128:(c + 1) * 128], c64[:],
                            skip_group_check=True)
                        nc.tensor.matmul(
                            stp[:, 8 + c:9 + c],
                            sr2_n[0:1, c * 128:(c + 1) * 128], c64[:],
                            skip_group_check=True)
                    nc.vector.tensor_copy(ST[:], stp[:, 0:16])

            # ---- phase B: num/den DoubleRow accumulation + epilogue ----
            with tc.tile_pool(name="phaseB", bufs=1) as pb:
                for u, (x, hp, q) in enumerate(U_LIST):
                    rb = hp * cc_rows[x]
                    eb = CH_PAIRS[x] * 128
                    nc.sync.dma_start(
                        ekg[u][:],
                        cc_out[x][rb + q * 128:rb + (q + 1) * 128, :])
                    nc.sync.dma_start(
                        ekvg[u][:], cc_out[x][rb + eb + q * 128:
                                              rb + eb + (q + 1) * 128, :])

                # S rows of both halves (raw fp32 bytes in the last chunk's
                # tail rows of each rank block) -> SBUF -> add
                gs = [pb.tile([1, D], F32, name=f"gs{i}", tag=f"gs{i}")
                      for i in range(4)]  # [d0, n0, d1, n1]
                for rk in range(2):
                    rb = rk * cc_rows[2] + 2 * CH_PAIRS[2] * 128
                    for ih in range(2):
                        sl = slice(ih * 512, (ih + 1) * 512)
                        nc.sync.dma_start(
                            gs[2 * rk][:, sl].bitcast(F8),
                            cc_out[2][rb + ih:rb + 1 + ih, :])
                        nc.sync.dma_start(
                            gs[2 * rk + 1][:, sl].bitcast(F8),
                            cc_out[2][rb + 2 + ih:rb + 3 + ih, :])
                nc.vector.tensor_add(sr2_d[:], gs[0][:], gs[2][:])
                nc.vector.tensor_add(sr2_n[:], gs[1][:], gs[3][:])

                yT = [pb.tile([128, TOK], BF16, name=f"yT{c}", tag=f"yT{c}")
                      for c in range(KT)]

                for c in range(KT):
                    cs = slice(c * 128, (c + 1) * 128)
                    pn = [psp.tile([128, 512], F32, name=f"pn{c}_{ih}",
                                   tag="mm") for ih in range(2)]
                    pd = [psp.tile([128, 512], F32, name=f"pd{c}_{ih}",
                                   tag="mm") for ih in range(2)]
                    for u in range(PAIRS):
                        for ih in range(2):
                            isl = slice(ih * 512, (ih + 1) * 512)
                            nc.tensor.matmul(
                                pn[ih][:], ekvg[u][:, :, cs],
                                pbr[u][:, :, isl],
                                start=(u == 0), stop=(u == PAIRS - 1),
                                perf_mode=DR)
                        for ih in range(2):
                            isl = slice(ih * 512, (ih + 1) * 512)
                            nc.tensor.matmul(
                                pd[ih][:], ekg[u][:, :, cs],
                                pbr[u][:, :, isl],
                                start=(u == 0), stop=(u == PAIRS - 1),
                                perf_mode=DR)
                    if c == 0:
                        emit_s_transpose()
                    for ih in range(2):
                        sl = slice(ih * 512, (ih + 1) * 512)
                        # num/den = (pn + 64*Snum) / (pd + 64*Sden): the
                        # einsum's 64x pre-scale and ST's 64x cancel.
                        den = pb.tile([128, 512], F32, name=f"den{c}{ih}",
                                      tag="den", bufs=3)
                        nc.scalar.activation(
                            den[:], pd[ih][:], AF.Identity,
                            bias=ST[:, c:c + 1])
                        rec = pb.tile([128, 512], F32, name=f"rec{c}{ih}",
                                      tag="rec", bufs=3)
                        nc.vector.reciprocal_approx_fast(rec[:], den[:])
                        tt = pb.tile([128, 512], F32, name=f"tt{c}{ih}",
                                     tag="tt", bufs=3)
                        nc.vector.scalar_tensor_tensor(
                            tt[:], pn[ih][:], ST[:, 8 + c:9 + c], rec[:],
                            ALU.add, ALU.mult)
                        nc.vector.tensor_mul(yT[c][:, sl], tt[:],
                                             sq_t[c][:, sl])

                # output projection: lhsT = y^T directly
                for it in range(NG):
                    po = [psp.tile([128, 512], F32, name=f"po{it}_{n}",
                                   tag="mm") for n in range(2)]
                    for c in range(KT):
                        for n in range(2):
                            nc.tensor.matmul(
                                po[n][:], yT[c][:, it * 128:(it + 1) * 128],
                                wout[c][:, n * 512:(n + 1) * 512],
                                start=(c == 0),
                                stop=(c == KT - 1 and not with_out_bias))
                    if with_out_bias:
                        for n in range(2):
                            nc.tensor.matmul(
                                po[n][:], ones1r[:],
                                bout[:, n * 512:(n + 1) * 512],
                                start=False, stop=True)
                    for n in range(2):
                        osb = pb.tile([128, 512], F32, name=f"osb{it}_{n}",
                                      tag="osb", bufs=4)
                        nc.scalar.copy(osb[:], po[n][:])
                        nc.sync.dma_start(
                            out_d[it * 128:(it + 1) * 128,
                                  n * 512:(n + 1) * 512], osb[:])

    nc.compile()
    return nc


def _prep_inputs(data, W_qkv, b_qkv, pos_bias_param, W_out, b_out):
    bf = ml_dtypes.bfloat16
    f8 = ml_dtypes.float8_e4m3
    data = np.asarray(data, np.float32)
    W_qkv = np.asarray(W_qkv, np.float32)
    b_qkv = np.asarray(b_qkv, np.float32)
    pos_bias_param = np.asarray(pos_bias_param, np.float32)
    W_out = np.asarray(W_out, np.float32)
    b_out = np.asarray(b_out, np.float32)

    with_qkv_bias = bool(np.any(b_qkv))
    with_out_bias = bool(np.any(b_out))

    wq = np.ascontiguousarray(W_qkv[:, :D]).astype(bf)
    wkv = np.ascontiguousarray(W_qkv[:, D:]).astype(bf)
    wout = W_out.astype(bf)
    # pbr[j, i] = expm1(pb[i, j]) * SCALE, fp8 (correction term of exp(pb))
    pbr_full = np.clip(np.expm1(pos_bias_param.T) * SCALE, -240.0, 240.0)
    pbr_full = pbr_full.astype(f8)

    in_maps = []
    for r in range(N_CORES):
        b, h = r // 2, r % 2
        isl = slice(h * TOK, (h + 1) * TOK)
        dT = np.ascontiguousarray(data[isl, b, :].T).astype(bf)  # [d_in, tok]
        # pair-block layout: rows u*128.. hold j-pair u; column halves are
        # the two 128-j subtiles (DoubleRow [128, 2, 1024])
        pbr_c = np.empty((TOK, 2048), f8)
        for u, (x, hp, q) in enumerate(U_LIST):
            J0 = hp * 1024 + (CH_START[x] + 2 * q) * 128
            pbr_c[u * 128:(u + 1) * 128, :TOK] = pbr_full[J0:J0 + 128, isl]
            pbr_c[u * 128:(u + 1) * 128, TOK:] = pbr_full[J0 + 128:J0 + 256, isl]
        m = {"dataT": dT, "wq": wq, "wkv": wkv, "pbr": pbr_c, "wout": wout}
        if with_qkv_bias:
            m["bkv"] = np.ascontiguousarray(b_qkv[D:]).reshape(1, 2 * D).astype(bf)
            m["bqt"] = np.ascontiguousarray(
                b_qkv[:D].reshape(KT, 128).T).astype(np.float32)
        if with_out_bias:
            m["bout"] = b_out.reshape(1, D).astype(bf)
        in_maps.append(m)
    return in_maps, with_qkv_bias, with_out_bias


def run(data, W_qkv, b_qkv, pos_bias_param, W_out, b_out, **spmd_kwargs):
    in_maps, wb, ob = _prep_inputs(data, W_qkv, b_qkv, pos_bias_param, W_out,
                                   b_out)
    key = (wb, ob)
    if key not in _cache:
        _cache[key] = build(wb, ob)
    nc = _cache[key]
    res = run_bass_kernel_spmd(nc, in_maps, core_ids=list(range(N_CORES)),
                               **spmd_kwargs)
    out = np.empty((T, B, D), np.float32)
    for r in range(N_CORES):
        b, h = r // 2, r % 2
        out[h * TOK:(h + 1) * TOK, b, :] = res.results[r]["out"]
    return out, res


def kernel(data, W_qkv, b_qkv, pos_bias_param, W_out, b_out):
    out, _ = run(data, W_qkv, b_qkv, pos_bias_param, W_out, b_out)
    return out
